# revision 42
# baseline (speedup 1.0000x reference)
"""Trainium2 Bass kernel for nn_BayesianOddLayer (GNN message passing).

Computation (per reference):
    total_mask = w_odd2even_mask * odd_weights              # [E, E]
    z          = (u < sigmoid(dropout_logits))              # [E]
    msg        = x @ (total_mask * z[:, None])              # [B, E]
    skip       = llr @ (w_skipconn2even_mask * llr_weights) # [B, E]
    out        = tanh(0.5 * clip(msg + skip, -10, 10))

Structure exploited: w_odd2even_mask[e1, e2] is nonzero only when
var(e1) == var(e2) (Tanner graph), and the skip term feeds each edge
from exactly its own variable.  The 512 variables are packed into 16
OUTPUT TILES of exactly 128 edges each (whole variables per tile), and
the tiles' variables into 4 VAR-TILES of <= 128 variables.  Each output
tile is then TWO accumulating matmuls into the same PSUM region:
    msg : lhsT = masked/z-gated ow block  [128 tile edges, 128 tile edges]
    skip: lhsT = masked lw block          [128 var-tile vars, 128 tile edges]
so every PSUM partition is a real output edge (a combined edges+vars
packing needs 20 partial tiles -> 25% wasted tanh + store traffic).
Within a group the 4 msg matmuls run back-to-back before the 4 skip
matmuls: consecutive matmuls never hit the same PSUM bank, so fill and
drain overlap fully (same-slice pairs serialize at +25%).

Engine budget per core: ACT tanh is the floor (16 groups x 2048
elems/lane at ~1/cycle @1.2GHz ~ 33us) with matmuls (~28us) hidden
under it.  The combined DMA fabric (~420 GB/s SBUF AXI, shared by
loads AND stores) binds at ~21MB of traffic, so the output is
quantized to int8 (t*127 on DVE, round-to-nearest, |err| <= 0.004 <<
the 2e-2 tolerance), halving store traffic.  GPSIMD elementwise is 35x
slower than DVE - never used.  All loads ride the sync HWDGE ring as
>=640KB transfers ((vt|rt) fused per group; weights packed with u/lg
into two ~1MB head DMAs interleaved with chunk-0 groups); running a
second ring during the ramp measurably THROTTLES the first (~300 GB/s
combined vs ~400 solo).  Stores: chunks 0-1 as single 1MB SWDGE DMAs
(small SWDGE stores sustain only ~110GB/s under load contention),
chunk 2 on the by-then-idle sync ring, last chunk per-group on
alternating rings for a fine-grained parallel drain.

Precision: matmul operands fp16 (|v| < 6; fp16 products exact in fp32
PSUM accumulate).  Dropout compare u < sigmoid(logits) in fp32.  tanh
on ACT from PSUM.  The +-10 clip is elided when a rigorous host-side
bound on the actual inputs shows it cannot bind.

Sharding: data-parallel over batch across 8 NeuronCores; weights
replicated.
"""

from contextlib import ExitStack

import numpy as np

import concourse.bass as bass
import concourse.mybir as mybir
from concourse import bacc
from concourse.bass_utils import run_bass_kernel_spmd
from concourse.tile import TileContext

F32 = mybir.dt.float32
F16 = mybir.dt.float16
I8 = mybir.dt.int8
AF = mybir.ActivationFunctionType
ALU = mybir.AluOpType

B = 16384  # batch
E = 2048  # edges
NV = 512  # variable nodes
NCORES = 8
BSH = B // NCORES  # batch rows per core
CHUNK = 512  # batch columns per matmul (hw limit on the moving operand)
NCHUNK = BSH // CHUNK
P = 128  # partitions
NT = E // P  # output tiles (16), each exactly 128 edges
NQ = NT // 4  # quads = ACT groups per chunk (4)
NWARM = 13  # PE warmup matmuls (bridge HAM until first real matmul)
QSCALE = 127.0  # int8 output quantization scale
WQ = 4 * (4 * P)  # weight cols per quad: [ew 512 | em 512 | sw 512 | sm 512]
UHD = 4 * NT  # u/lg header cols (fp32 bit patterns as fp16 pairs)
GW = 5 * CHUNK  # rhs cols per (chunk, group): [vt 512 | rt 2048]


def _plan_tiles(w_skipconn2even_mask: np.ndarray):
    """Pack whole variables into NT tiles of exactly P edges each, and the
    tiles' variables into NQ var-tiles of <= P variables (tile t's vars
    live in var-tile t//4).

    Returns (tile_edges [NT][P], vtile_vars [NQ][<=P]).
    """
    var = w_skipconn2even_mask.argmax(axis=0).astype(np.int64)  # [E]
    deg = np.bincount(var, minlength=NV)
    vars_nz = np.where(deg > 0)[0]
    order = vars_nz[np.argsort(-deg[vars_nz], kind="stable")]
    gsum = np.zeros(NT, np.int64)
    gnv = np.zeros(NT, np.int64)
    groups = [[] for _ in range(NT)]
    for v in order:
        dv = int(deg[v])
        cand = [g for g in range(NT) if gsum[g] + dv <= P]
        assert cand, "greedy packing failed"
        g = min(cand, key=lambda g: (gsum[g], gnv[g]))
        groups[g].append(int(v))
        gsum[g] += dv
        gnv[g] += 1
    assert all(s == P for s in gsum), f"imperfect packing {gsum}"

    # assign the 16 groups to 4 var-tiles (4 each), balancing #vars <= P
    tile_nv = np.zeros(NQ, np.int64)
    tile_cnt = np.zeros(NQ, np.int64)
    assign = [[] for _ in range(NQ)]
    for g in np.argsort(-gnv, kind="stable"):
        q = min(
            [q for q in range(NQ) if tile_cnt[q] < 4], key=lambda q: tile_nv[q]
        )
        assign[q].append(int(g))
        tile_nv[q] += gnv[g]
        tile_cnt[q] += 1
    assert all(n <= P for n in tile_nv), f"var-tile overflow {tile_nv}"

    edges_of = {v: np.where(var == v)[0] for v in vars_nz}
    tile_edges = []
    vtile_vars = []
    for q in range(NQ):
        vlist = []
        for g in assign[q]:
            gv = sorted(groups[g])
            vlist.extend(gv)
            te = np.concatenate([edges_of[v] for v in gv])
            assert te.size == P
            tile_edges.append(te)
        vtile_vars.append(np.array(vlist))
    assert sum(t.size for t in tile_edges) == E
    return tile_edges, vtile_vars


def _build_nc(need_clamp):
    nc = bacc.Bacc("TRN2", target_bir_lowering=False, debug=False,
                   num_devices=NCORES)
    W = NT * CHUNK  # out free-dim per chunk
    RW = NQ * GW  # rhs free-dim per chunk
    wcomb = nc.dram_tensor(
        "wcomb", [P, UHD + NQ * WQ], F16, kind="ExternalInput").ap()
    rhsp = nc.dram_tensor("rhsp", [P, NCHUNK * RW], F16, kind="ExternalInput").ap()
    outp = nc.dram_tensor("outp", [P, NCHUNK * W], I8, kind="ExternalOutput").ap()
    # the very last group stores fp16 directly (its int8 convert would sit
    # on the critical tail after the final tanh)
    outp16 = nc.dram_tensor("outp16", [P, 4 * CHUNK], F16, kind="ExternalOutput").ap()

    with TileContext(nc) as tc, ExitStack() as ctx:
        cpool = ctx.enter_context(tc.tile_pool(name="const", bufs=1))
        rpool = ctx.enter_context(tc.tile_pool(name="rhs", bufs=6))
        opool = ctx.enter_context(tc.tile_pool(name="out", bufs=4))
        o8pool = ctx.enter_context(tc.tile_pool(name="out8", bufs=4))
        pspool = ctx.enter_context(tc.tile_pool(name="ps", bufs=2, space="PSUM"))

        # PE warmup operands: memset first on gpsimd so warmups run during
        # the initial DMA window and release the HAM clock gate (1.2->2.4GHz)
        zl = cpool.tile([P, P], F16)
        nc.gpsimd.memset(zl[:], 0.0)
        zr = cpool.tile([P, CHUNK], F16)
        nc.gpsimd.memset(zr[:], 0.0)

        # u/lg rides its own tiny first DMA: the z-chain (copy, sigmoid,
        # is_lt) gates ALL weight prep, and a small transfer's completion
        # receipt fires ~2-3us earlier than the 1MB head's.  Then
        # [wq0 | wq1] (gates groups 0-1) and [wq2 | wq3] between the
        # group-1 and group-2 rhs loads (separate tiles: a shared tile
        # would add a WAR hazard with the group-0/1 matmul reads)
        ut = cpool.tile([P, UHD], F16)
        nc.sync.dma_start(ut[:], wcomb[:, 0:UHD])
        wh01 = cpool.tile([P, 2 * WQ], F16)
        nc.sync.dma_start(wh01[:], wcomb[:, UHD : UHD + 2 * WQ])
        # wq2/wq3 ride the otherwise-idle gpsimd ring: keeps their 1MB off
        # the sync ring ahead of the group-1 rhs (they are not needed until
        # ~16us, and unlike shipping ALL weights via SWDGE this transfer is
        # small enough not to strangle the sync ring)
        wh23 = cpool.tile([P, 2 * WQ], F16)
        nc.gpsimd.dma_start(wh23[:], wcomb[:, UHD + 2 * WQ :])

        def wt(q):
            if q < 2:
                return wh01[:, q * WQ : (q + 1) * WQ]
            return wh23[:, (q - 2) * WQ : (q - 1) * WQ]

        # z = (u < sigmoid(dropout_logits)) in fp32 (u/logits arrive as raw
        # fp32 bit patterns packed in the fp16 tensor; DVE copy feeds ACT a
        # clean f32 tile since ACT cannot take bitcast APs)
        zt = cpool.tile([P, NT], F32)
        nc.vector.tensor_copy(zt[:], ut[:, 2 * NT : 4 * NT].bitcast(F32))
        nc.scalar.activation(zt[:], zt[:], AF.Sigmoid)
        nc.vector.tensor_tensor(
            zt[:], ut[:, 0 : 2 * NT].bitcast(F32), zt[:], ALU.is_lt)

        wps = pspool.tile([P, 4 * CHUNK], F32, tag="ps")
        for _ in range(NWARM):
            nc.tensor.matmul(wps[:, 0:CHUNK], zl[:], zr[:], start=True, stop=True)

        # weight prep (all DVE; gpsimd elementwise is 35x slower):
        # edge blocks (w * z[src edge]) * mask fused per tile; skip blocks
        # one w*mask per quad
        def prep_quad(q):
            w = wt(q)
            for i in range(4):
                t = 4 * q + i
                sl = w[:, i * P : (i + 1) * P]
                nc.vector.scalar_tensor_tensor(
                    sl, sl, zt[:, t : t + 1],
                    w[:, 4 * P + i * P : 4 * P + (i + 1) * P],
                    ALU.mult, ALU.mult)
            nc.vector.tensor_tensor(
                w[:, 8 * P : 12 * P], w[:, 8 * P : 12 * P],
                w[:, 12 * P : 16 * P], ALU.mult)

        for nb in range(NCHUNK):
            for q in range(NQ):
                if nb == 0 and q == 0:
                    prep_quad(0)
                    prep_quad(1)
                rtv = rpool.tile([P, GW], F16)
                c0 = nb * RW + q * GW
                nc.sync.dma_start(rtv[:], rhsp[:, c0 : c0 + GW])
                if nb == 0 and q == 0:
                    # ALL prep ops are emitted before any int8 convert so
                    # the DVE FIFO never holds group 2-3 prep hostage to a
                    # convert that waits on a later tanh
                    prep_quad(2)
                    prep_quad(3)
                vt = rtv[:, 0:CHUNK]
                rt = rtv[:, CHUNK:GW]
                ps = pspool.tile([P, 4 * CHUNK], F32)
                # msg matmuls first, then the accumulating skip matmuls
                for i in range(4):
                    nc.tensor.matmul(
                        ps[:, i * CHUNK : (i + 1) * CHUNK],
                        wt(q)[:, i * P : (i + 1) * P],
                        rt[:, i * CHUNK : (i + 1) * CHUNK],
                        start=True, stop=False)
                for i in range(4):
                    nc.tensor.matmul(
                        ps[:, i * CHUNK : (i + 1) * CHUNK],
                        wt(q)[:, (8 + i) * P : (9 + i) * P], vt,
                        start=False, stop=True)
                last = nb == NCHUNK - 1 and q == NQ - 1
                ot = opool.tile([P, 4 * CHUNK], F16)
                if need_clamp:
                    nc.vector.tensor_scalar(
                        ot[:], ps[:], 10.0, -10.0, ALU.min, ALU.max)
                    nc.scalar.activation(ot[:], ot[:], AF.Tanh, scale=0.5)
                elif last:
                    # final group in two halves: the first half's store
                    # drains while the second half's tanh still runs
                    h = 2 * CHUNK
                    nc.scalar.activation(
                        ot[:, 0:h], ps[:, 0:h], AF.Tanh, scale=0.5)
                    nc.sync.dma_start(outp16[:, 0:h], ot[:, 0:h])
                    nc.scalar.activation(
                        ot[:, h:], ps[:, h:], AF.Tanh, scale=0.5)
                else:
                    # clip(v, +-10) proven identity for these inputs (host
                    # bound); tanh straight from PSUM
                    nc.scalar.activation(ot[:], ps[:], AF.Tanh, scale=0.5)
                # int8 quantize on DVE (round-to-nearest, ~1us per group)
                if nb < NCHUNK - 1:
                    # chunks 0-2: quantize into a per-chunk tile, store as
                    # ONE 1MB DMA (small SWDGE stores sustain only ~110GB/s
                    # under load contention); chunk 2 rides the sync ring,
                    # idle once loads finish
                    if q == 0:
                        o8c = o8pool.tile([P, NT * CHUNK], I8, tag="o8c", bufs=2)
                    nc.vector.tensor_scalar(
                        o8c[:, q * 4 * CHUNK : (q + 1) * 4 * CHUNK],
                        ot[:], QSCALE, None, ALU.mult)
                    if q == NQ - 1:
                        eng = nc.sync if nb == 2 else nc.gpsimd
                        eng.dma_start(outp[:, nb * W : (nb + 1) * W], o8c[:])
                elif q < NQ - 1:
                    # last chunk: per-group stores alternating rings so the
                    # final drain is parallel and fine-grained
                    o8 = o8pool.tile([P, 4 * CHUNK], I8)
                    nc.vector.tensor_scalar(o8[:], ot[:], QSCALE, None, ALU.mult)
                    c0 = nb * W + q * 4 * CHUNK
                    if q % 2 == 0:
                        nc.gpsimd.dma_start(outp[:, c0 : c0 + 4 * CHUNK], o8[:])
                    else:
                        nc.sync.dma_start(outp[:, c0 : c0 + 4 * CHUNK], o8[:])
                elif need_clamp:
                    nc.sync.dma_start(outp16[:], ot[:])
                else:
                    # final group: fp16 straight out, no convert on the
                    # tail (first half's store was issued above)
                    nc.sync.dma_start(
                        outp16[:, 2 * CHUNK :], ot[:, 2 * CHUNK :])
    nc.compile()
    return nc


def _prep(x, llr, u, odd_weights, llr_weights, dropout_logits,
          w_odd2even_mask, w_skipconn2even_mask):
    """Host-side data movement: tile packing, block gathers, shards, casts."""
    ow = np.asarray(odd_weights, np.float32)
    msk = np.asarray(w_odd2even_mask, np.float32)
    lw = np.asarray(llr_weights, np.float32)
    smask = np.asarray(w_skipconn2even_mask, np.float32)
    u = np.asarray(u, np.float32)
    lg = np.asarray(dropout_logits, np.float32)

    tile_edges, vtile_vars = _plan_tiles(smask)

    wblk = np.zeros((P, NQ * WQ), np.float16)
    ucomb = np.zeros((P, NT), np.float32)
    lgcomb = np.zeros((P, NT), np.float32)
    for t in range(NT):
        q = t // 4
        i = t % 4
        pe = tile_edges[t]
        vs = vtile_vars[q]
        c = q * WQ
        wblk[:, c + i * P : c + (i + 1) * P] = ow[np.ix_(pe, pe)].astype(np.float16)
        wblk[:, c + (4 + i) * P : c + (5 + i) * P] = msk[np.ix_(pe, pe)].astype(np.float16)
        wblk[: vs.size, c + (8 + i) * P : c + (9 + i) * P] = lw[np.ix_(vs, pe)].astype(np.float16)
        wblk[: vs.size, c + (12 + i) * P : c + (13 + i) * P] = smask[np.ix_(vs, pe)].astype(np.float16)
        ucomb[:, t] = u[pe]
        lgcomb[:, t] = lg[pe]

    x = np.asarray(x, np.float32)
    llr = np.asarray(llr, np.float32)

    # Rigorous bound on |msg + skip|: if it cannot reach the +-10 clip,
    # the clip is the identity and the device clamp stage is elided.
    xmax = float(np.abs(x).max())
    lmax = float(np.abs(llr).max())
    wf = wblk.astype(np.float32).reshape(P, NQ, 16, P)
    awe = np.abs(wf[:, :, 0:4] * wf[:, :, 4:8])  # |ow*mask| per tile
    aws = np.abs(wf[:, :, 8:12] * wf[:, :, 12:16])
    bound = float((awe.sum(axis=0) * xmax + aws.sum(axis=0) * lmax).max())
    need_clamp = bound >= 9.5

    # head: u/logits as raw fp32 bit patterns viewed as fp16 pairs, then
    # the four weight quads
    wcomb = np.ascontiguousarray(np.concatenate(
        [ucomb.view(np.float16), lgcomb.view(np.float16), wblk], axis=1))
    assert wcomb.shape == (P, UHD + NQ * WQ)

    # rhs row ids per chunk: per group q, its var-tile then its 4 edge-tiles
    rows = np.full(NQ * 5 * P, E + NV, np.int64)
    for q in range(NQ):
        vs = vtile_vars[q]
        rows[q * 5 * P : q * 5 * P + vs.size] = E + vs
        for i in range(4):
            t = 4 * q + i
            rows[(q * 5 + 1 + i) * P : (q * 5 + 2 + i) * P] = tile_edges[t]

    in_maps = []
    for c in range(NCORES):
        sl = slice(c * BSH, (c + 1) * BSH)
        base = np.concatenate(
            [x[sl].T, llr[sl].T, np.zeros((1, BSH), np.float32)], axis=0
        ).astype(np.float16)
        rhs = base[rows]  # [NQ*5*P, BSH] fp16
        rhsp = np.ascontiguousarray(
            rhs.reshape(NQ * 5, P, NCHUNK, CHUNK).transpose(1, 2, 0, 3)
        ).reshape(P, NCHUNK * NQ * 5 * CHUNK)
        in_maps.append({
            "wcomb": wcomb,
            "rhsp": rhsp,
        })
    return tile_edges, in_maps, need_clamp


def _run(inputs: dict, trace: bool = False, **kwargs):
    tile_edges, in_maps, need_clamp = _prep(**inputs)
    nc = _build_nc(need_clamp)
    res = run_bass_kernel_spmd(nc, in_maps, list(range(NCORES)), trace=trace, **kwargs)

    dest = np.concatenate(tile_edges)  # row (t, p) -> edge column
    out = np.empty((B, E), np.float32)
    for c in range(NCORES):
        sl = slice(c * BSH, (c + 1) * BSH)
        a8 = (res.results[c]["outp"]
              .reshape(P, NCHUNK, NT, CHUNK)
              .astype(np.float32) * np.float32(1.0 / QSCALE))
        # last group (chunk 3, tiles 12-15) arrived as raw fp16
        a16 = (res.results[c]["outp16"]
               .reshape(P, 4, CHUNK)
               .astype(np.float32))
        a8[:, NCHUNK - 1, NT - 4 : NT, :] = a16
        arr = a8.transpose(2, 0, 1, 3).reshape(NT * P, BSH)
        out[sl][:, dest] = arr.T
    return out, res


def kernel(**inputs) -> np.ndarray:
    out, _ = _run(inputs, trace=False)
    return out
